# revision 12
# baseline (speedup 1.0000x reference)
"""Trainium2 Bass kernel for nn_Decoder (2-layer LSTM-with-projection decoder).

Math (per batch element, serial chain over 12000 = 1000 nodes x 12 timesteps):
    x      = [enc_n, last + node_mean_n]          (last = h2 of prev step, s0 at t=0)
    h1,c1  = LSTMCell_proj(x, h1, c1; W*_l0, W_hr_l0)
    h2,c2  = LSTMCell_proj(h1, h2, c2; W*_l1, W_hr_l1)
    out    = h2 + node_mean_n

Distribution: data-parallel over batch B=64 -> 8 cores x 8 (sharding hint).
Layout on device (per core): "gate-on-partition": all state tiles are
[128 partitions, 8 batch] so every engine op uses full-width lanes and all
contractions (gates K<=128, projection K=128) land on the partition axis
without any transposes.

The chain is strictly serial, so the program is emitted FULLY UNROLLED in raw
Bass (no Tile framework, no loops): monotonically increasing semaphore
immediates encode the exact cross-engine dependency chain.  One semaphore per
producer engine; every compute instruction incs its engine's semaphore by 1,
so "wait sem_X >= k" means "engine X finished its first k instructions".

X rhs tile layout (partitions; engine APs must start 32-aligned):
    0:16   h2T (aka "last"; s0 at start, h02 swapped in after step (0,0))
    16     ones row (folds biases into the gate matmuls)
    17:32  zeros
    32:48  h1T
    48:64  node_mean SUM over t (x W/12 folded into weights); DMA-written
    64:128 encT for current node
  gates1 = W1.T @ X[0:128],  gates2 = W2.T @ X[0:64]   (4 MMs each, M=128)
"""

import numpy as np

_B, _N, _FUT, _E, _H, _O = 64, 1000, 12, 64, 128, 16
_NCORES, _BS = 8, 8

_cache = {}


def _dma_plan(n_nodes):
    order = [("w1",), ("w2",), ("wr1",), ("wr2",),
             ("sT",), ("ones0",), ("h01",), ("c1",), ("c2",),
             ("ones1",), ("z0",), ("z1",), ("M", 0)]
    if n_nodes > 1:
        order.append(("M", 1))
    order += [("enc", 0), ("NM", 0), ("h02",)]
    for n in range(n_nodes):
        if n + 1 < n_nodes:
            order.append(("enc", n + 1))
        if n + 2 < n_nodes:
            order.append(("M", n + 2))
        if n + 1 < n_nodes:
            order.append(("NM", n + 1))
        order.append(("OUT", n))
    return {tag: 16 * (i + 1) for i, tag in enumerate(order)}


def _build(n_nodes, fut=_FUT, debug=False):
    import concourse.bass as bass
    import concourse.mybir as mybir
    from contextlib import ExitStack

    f32 = mybir.dt.float32
    AF = mybir.ActivationFunctionType
    AL = mybir.AluOpType

    FUT = fut
    ACT_PER_NODE = 8 * FUT + 1
    DVE_PER_NODE = 9 * FUT + 1

    # ---- semaphore-count index helpers (1-based: value AFTER instr) ----
    def pe_idx(s, j):           # j in 1..10
        return 10 * s + j

    def act_idx(n, t, name):
        b = ACT_PER_NODE * n + 8 * t
        if t < FUT - 1:
            j = {"sig1": 1, "tg1": 2, "tc1": 3, "cp1": 4,
                 "sig2": 5, "tg2": 6, "tc2": 7, "cp2": 8}[name]
        else:
            j = {"sig1": 1, "tg1": 2, "tc1": 3, "cp1": 4, "cp1b": 5,
                 "sig2": 6, "tg2": 7, "tc2": 8, "cp2": 9}[name]
        return b + j

    def dve_base(n):
        return 1 + DVE_PER_NODE * n

    def dve_has_red(n):
        return 1 if n < n_nodes - 1 else 0

    def dve_idx(n, t, j):       # j in 1..9
        if t == 0:
            return dve_base(n) + j
        return dve_base(n) + dve_has_red(n) + 9 * t + j

    def dve_reduce_idx(k):
        # reduce(k) runs right after step 0 of node k-1 (k>=1); reduce(0) is first
        return 1 if k == 0 else dve_base(k - 1) + 9 + 1

    def rdy_node(m):            # s_rdy value after node-m group fence
        return 4 + m

    dmav = _dma_plan(n_nodes)

    nc = bass.Bass()
    enc_d = nc.dram_tensor("enc", [n_nodes * 64, _BS], f32, kind="ExternalInput")
    mean_d = nc.dram_tensor("meanx", [n_nodes * 16, 8 * FUT], f32, kind="ExternalInput")
    w1_d = nc.dram_tensor("w1", [128, 512], f32, kind="ExternalInput")
    w2_d = nc.dram_tensor("w2", [64, 512], f32, kind="ExternalInput")
    wr1_d = nc.dram_tensor("wr1", [128, 16], f32, kind="ExternalInput")
    wr2_d = nc.dram_tensor("wr2", [128, 16], f32, kind="ExternalInput")
    sT_d = nc.dram_tensor("sT", [16, _BS], f32, kind="ExternalInput")
    h01_d = nc.dram_tensor("h01", [16, _BS], f32, kind="ExternalInput")
    h02_d = nc.dram_tensor("h02", [16, _BS], f32, kind="ExternalInput")
    c1_d = nc.dram_tensor("c1i", [128, _BS], f32, kind="ExternalInput")
    c2_d = nc.dram_tensor("c2i", [128, _BS], f32, kind="ExternalInput")
    one_d = nc.dram_tensor("onesr", [1, _BS], f32, kind="ExternalInput")
    z_d = nc.dram_tensor("zerosr", [15, _BS], f32, kind="ExternalInput")
    out_d = nc.dram_tensor("outp", [n_nodes * 16, 8 * FUT], f32, kind="ExternalOutput")
    if debug:
        dbg_names = ["dS1a", "dS1b", "dS2a", "dS2b", "dC1", "dC2", "dX0", "dX1",
                     "dNMa", "dHF1", "dHF2", "dTC1", "dTC2"]
        dbg_d = {nm: nc.dram_tensor(nm, [128, 32], f32, kind="ExternalOutput")
                 for nm in dbg_names}

    ctx = ExitStack()
    sb = lambda nm, shape: ctx.enter_context(nc.sbuf_tensor(nm, shape, f32))
    ps = lambda nm, shape: ctx.enter_context(nc.psum_tensor(nm, shape, f32))

    X = [sb("Xt0", [128, _BS]), sb("Xt1", [128, _BS])]
    W1s, W2s = sb("W1s", [128, 512]), sb("W2s", [64, 512])
    WR1, WR2 = sb("WR1", [128, 16]), sb("WR2", [128, 16])
    C1, C2 = sb("C1t", [128, _BS]), sb("C2t", [128, _BS])
    S1 = [sb("S1a", [128, 32]), sb("S1b", [128, 32])]
    S2 = [sb("S2a", [128, 32]), sb("S2b", [128, 32])]
    T1, T2 = sb("T1t", [128, _BS]), sb("T2t", [128, _BS])
    TC1, TC2 = sb("TC1", [128, _BS]), sb("TC2", [128, _BS])
    HF1, HF2 = sb("HF1", [128, _BS]), sb("HF2", [128, _BS])
    M = [sb("Ma", [16, 8 * FUT]), sb("Mb", [16, 8 * FUT])]
    NM = [sb("NMa", [16, _BS]), sb("NMb", [16, _BS])]
    OUT = [sb("OUTa", [16, 8 * FUT]), sb("OUTb", [16, 8 * FUT])]
    G1 = [ps("G1a", [128, 32]), ps("G1b", [128, 32])]
    G2 = [ps("G2a", [128, 32]), ps("G2b", [128, 32])]
    P1, P2 = ps("P1t", [16, _BS]), ps("P2t", [16, _BS])

    s_pe = ctx.enter_context(nc.semaphore("s_pe"))
    s_act = ctx.enter_context(nc.semaphore("s_act"))
    s_dve = ctx.enter_context(nc.semaphore("s_dve"))
    s_dma = ctx.enter_context(nc.semaphore("s_dma"))
    s_rdy = ctx.enter_context(nc.semaphore("s_rdy"))

    block = ctx.enter_context(nc.Block())

    @block.gpsimd
    def _(g):
        issued = [0]

        def dma(out_ap, in_ap, wait=None):
            if wait is not None:
                g.wait_ge(*wait)
            g.dma_start(out_ap, in_ap).then_inc(s_dma, 16)
            issued[0] += 1

        def fence():
            # s_dma only counts COMPLETIONS (order across queues is not
            # guaranteed), so the only safe wait is "everything issued so far
            # is done"; then publish a monotone ready-count on s_rdy.
            g.wait_ge(s_dma, 16 * issued[0])
            g.nop().then_inc(s_rdy, 1)

        dma(W1s[:, :], w1_d[:, :])
        dma(W2s[:, :], w2_d[:, :])
        dma(WR1[:, :], wr1_d[:, :])
        dma(WR2[:, :], wr2_d[:, :])
        dma(X[0][0:16, :], sT_d[:, :])
        dma(X[0][16:17, :], one_d[:, :])
        dma(X[0][32:48, :], h01_d[:, :])
        dma(C1[:, :], c1_d[:, :])
        dma(C2[:, :], c2_d[:, :])
        dma(X[1][16:17, :], one_d[:, :])
        dma(X[0][17:32, :], z_d[:, :])
        dma(X[1][17:32, :], z_d[:, :])
        dma(M[0][:, :], mean_d[0:16, :])
        if n_nodes > 1:
            dma(M[1][:, :], mean_d[16:32, :])
        fence()                                        # s_rdy = 1: weights+inits+M(0,1)
        dma(X[0][64:128, :], enc_d[0:64, :])
        dma(X[0][48:64, :], NM[0][:, :], wait=(s_dve, dve_reduce_idx(0)))
        fence()                                        # s_rdy = 2: enc(0), NM(0)
        dma(X[0][0:16, :], h02_d[:, :], wait=(s_pe, pe_idx(0, 4)))
        fence()                                        # s_rdy = 3: h02

        for n in range(n_nodes):
            if n + 1 < n_nodes:
                w = None if n == 0 else (s_pe, pe_idx(FUT * n - 1, 9))
                dma(X[(n + 1) % 2][64:128, :],
                    enc_d[(n + 1) * 64:(n + 2) * 64, :], wait=w)
            if n + 2 < n_nodes:
                dma(M[n % 2][:, :],
                    mean_d[(n + 2) * 16:(n + 3) * 16, :],
                    wait=(s_dve, dve_reduce_idx(n)))
            if n + 1 < n_nodes:
                dma(X[(n + 1) % 2][48:64, :], NM[(n + 1) % 2][:, :],
                    wait=(s_dve, dve_reduce_idx(n + 1)))
            fence()                                    # s_rdy = 4 + n
            dma(out_d[n * 16:(n + 1) * 16, :], OUT[n % 2][:, :],
                wait=(s_dve, dve_idx(n, FUT - 1, 9)))

        if debug:
            g.wait_ge(s_pe, 10 * FUT * n_nodes)
            g.wait_ge(s_act, ACT_PER_NODE * n_nodes)
            g.wait_ge(s_dve, dve_idx(n_nodes - 1, FUT - 1, 9))
            pairs = [("dS1a", S1[0][:, :]),
                     ("dS2a", S2[0][:, :]),
                     ("dC1", C1[:, :]), ("dC2", C2[:, :]),
                     ("dX0", X[0][:, :]),
                     ("dX1", X[1][0:48, :] if n_nodes == 1 else X[1][:, :]),
                     ("dNMa", NM[0][:, :]), ("dHF1", HF1[:, :]),
                     ("dHF2", HF2[:, :]), ("dTC1", TC1[:, :]),
                     ("dTC2", TC2[:, :])]
            if FUT * n_nodes >= 2:
                pairs += [("dS1b", S1[1][:, :]), ("dS2b", S2[1][:, :])]
            for nm, ap in pairs:
                h, w_ = ap.shape[0], ap.shape[1]
                g.dma_start(dbg_d[nm][0:h, 0:w_], ap).then_inc(s_dma, 16)

    @block.vector
    def _(v):
        def op(inst, wait=None):
            if wait is not None:
                inst._wait_ge(*wait)
            inst.then_inc(s_dve, 1)

        op(nc.vector.tensor_reduce(
            NM[0][:, :], M[0][:, :].rearrange("p (b t) -> p b t", t=FUT),
            axis=mybir.AxisListType.X, op=AL.add),
            wait=(s_rdy, 1))

        def emit_reduce(k):
            rw = 1 if k == 1 else rdy_node(k - 2)
            op(nc.vector.tensor_reduce(
                NM[k % 2][:, :],
                M[k % 2][:, :].rearrange("p (b t) -> p b t", t=FUT),
                axis=mybir.AxisListType.X, op=AL.add),
                wait=(s_rdy, rw))

        for n in range(n_nodes):
            if n >= 2:
                v.wait_ge(s_rdy, rdy_node(n - 1))   # covers OUT(n-2) dma
            for t in range(FUT):
                if t == 1 and n + 1 < n_nodes:
                    emit_reduce(n + 1)   # off the critical path, after step 0
                s = FUT * n + t
                p = s % 2
                op(nc.vector.tensor_mul(T1[:, :], S1[p][:, 0:8], C1[:, :]),
                   wait=(s_act, act_idx(n, t, "sig1")))
                op(nc.vector.tensor_mul(T2[:, :], S1[p][:, 8:16], S1[p][:, 24:32]),
                   wait=(s_act, act_idx(n, t, "tg1")))
                # self-wait: DVE RAW through T1/T2 is a pipeline hazard
                op(nc.vector.tensor_add(C1[:, :], T1[:, :], T2[:, :]),
                   wait=(s_dve, dve_idx(n, t, 2)))
                op(nc.vector.tensor_mul(HF1[:, :], S1[p][:, 16:24], TC1[:, :]),
                   wait=(s_act, act_idx(n, t, "tc1")))
                op(nc.vector.tensor_mul(T1[:, :], S2[p][:, 0:8], C2[:, :]),
                   wait=(s_act, act_idx(n, t, "sig2")))
                op(nc.vector.tensor_mul(T2[:, :], S2[p][:, 8:16], S2[p][:, 24:32]),
                   wait=(s_act, act_idx(n, t, "tg2")))
                op(nc.vector.tensor_add(C2[:, :], T1[:, :], T2[:, :]),
                   wait=(s_dve, dve_idx(n, t, 6)))
                op(nc.vector.tensor_mul(HF2[:, :], S2[p][:, 16:24], TC2[:, :]),
                   wait=(s_act, act_idx(n, t, "tc2")))
                op(nc.vector.scalar_tensor_tensor(
                    OUT[n % 2][:, 8 * t:8 * t + 8], NM[n % 2][:, :], 1.0 / FUT,
                    P2[:, :], AL.mult, AL.add),
                   wait=(s_pe, pe_idx(s, 10)))
            if FUT == 1 and n + 1 < n_nodes:
                emit_reduce(n + 1)

    @block.scalar
    def _(sc):
        def op(inst, wait=None):
            if wait is not None:
                inst._wait_ge(*wait)
            inst.then_inc(s_act, 1)

        for n in range(n_nodes):
            Xc = X[n % 2]
            Xn = X[(n + 1) % 2]
            for t in range(FUT):
                s = FUT * n + t
                p = s % 2
                op(nc.scalar.activation(S1[p][:, 0:24], G1[p][:, 0:24], AF.Sigmoid),
                   wait=(s_pe, pe_idx(s, 4)))
                op(nc.scalar.activation(S1[p][:, 24:32], G1[p][:, 24:32], AF.Tanh))
                op(nc.scalar.activation(TC1[:, :], C1[:, :], AF.Tanh),
                   wait=(s_dve, dve_idx(n, t, 3)))
                op(nc.scalar.copy(Xc[32:48, :] if t < FUT - 1 else Xc[32:48, :], P1[:, :]),
                   wait=(s_pe, pe_idx(s, 5)))
                if t == FUT - 1:
                    op(nc.scalar.copy(Xn[32:48, :], P1[:, :]),
                       wait=(s_pe, pe_idx(s, 5)))
                op(nc.scalar.activation(S2[p][:, 0:24], G2[p][:, 0:24], AF.Sigmoid),
                   wait=(s_pe, pe_idx(s, 9)))
                op(nc.scalar.activation(S2[p][:, 24:32], G2[p][:, 24:32], AF.Tanh))
                op(nc.scalar.activation(TC2[:, :], C2[:, :], AF.Tanh),
                   wait=(s_dve, dve_idx(n, t, 7)))
                op(nc.scalar.copy((Xc if t < FUT - 1 else Xn)[0:16, :], P2[:, :]),
                   wait=(s_pe, pe_idx(s, 10)))

    @block.tensor
    def _(te):
        def op(inst, wait=None):
            if wait is not None:
                inst._wait_ge(*wait)
            inst.then_inc(s_pe, 1)

        for n in range(n_nodes):
            Xc = X[n % 2]
            te.wait_ge(s_rdy, 2 if n == 0 else rdy_node(n - 1))
            for t in range(FUT):
                s = FUT * n + t
                p = s % 2
                for q in range(4):
                    w = None
                    if q == 0 and s >= 1:
                        pn, pt = divmod(s - 1, FUT)
                        w = (s_act, act_idx(pn, pt, "cp2"))
                    op(nc.tensor.matmul(G1[p][:, 8 * q:8 * q + 8],
                                        W1s[:, 128 * q:128 * q + 128],
                                        Xc[0:128, :], start=True, stop=True), wait=w)
                op(nc.tensor.matmul(P1[:, :], WR1[:, :], HF1[:, :],
                                    start=True, stop=True),
                   wait=(s_dve, dve_idx(n, t, 4)))
                if s == 0:
                    te.wait_ge(s_rdy, 3)
                for q in range(4):
                    w = (s_act, act_idx(n, t, "cp1")) if q == 0 else None
                    op(nc.tensor.matmul(G2[p][:, 8 * q:8 * q + 8],
                                        W2s[0:64, 128 * q:128 * q + 128],
                                        Xc[0:64, :], start=True, stop=True), wait=w)
                op(nc.tensor.matmul(P2[:, :], WR2[:, :], HF2[:, :],
                                    start=True, stop=True),
                   wait=(s_dve, dve_idx(n, t, 8)))

    ctx.close()
    return nc


def _prep_inputs(enc_outputs, mean, s, h_0, c_0,
                 W_ih_l0, W_hh_l0, b_ih_l0, b_hh_l0, W_hr_l0,
                 W_ih_l1, W_hh_l1, b_ih_l1, b_hh_l1, W_hr_l1, n_nodes,
                 fut=_FUT):
    f32 = np.float32
    # torch gate order i,f,g,o; PSUM column-block order f,i,o,g
    gate_slices = [slice(128, 256), slice(0, 128), slice(384, 512), slice(256, 384)]

    w1 = np.zeros((128, 512), f32)
    b1 = np.asarray(b_ih_l0, f32) + np.asarray(b_hh_l0, f32)
    for qi, R in enumerate(gate_slices):
        c = slice(128 * qi, 128 * qi + 128)
        w1[0:16, c] = np.asarray(W_ih_l0, f32)[R, 64:80].T
        w1[16, c] = b1[R]
        w1[32:48, c] = np.asarray(W_hh_l0, f32)[R, :].T
        w1[48:64, c] = np.asarray(W_ih_l0, f32)[R, 64:80].T / float(fut)
        w1[64:128, c] = np.asarray(W_ih_l0, f32)[R, 0:64].T

    w2 = np.zeros((64, 512), f32)
    b2 = np.asarray(b_ih_l1, f32) + np.asarray(b_hh_l1, f32)
    for qi, R in enumerate(gate_slices):
        c = slice(128 * qi, 128 * qi + 128)
        w2[0:16, c] = np.asarray(W_hh_l1, f32)[R, :].T
        w2[16, c] = b2[R]
        w2[32:48, c] = np.asarray(W_ih_l1, f32)[R, :].T

    wr1 = np.ascontiguousarray(np.asarray(W_hr_l0, f32).T)
    wr2 = np.ascontiguousarray(np.asarray(W_hr_l1, f32).T)

    ones8 = np.ones((1, _BS), f32)
    zeros8 = np.zeros((15, _BS), f32)
    sT = np.repeat(np.asarray(s, f32)[0][:, None], _BS, 1)
    h01 = np.repeat(np.asarray(h_0, f32)[0][:, None], _BS, 1)
    h02 = np.repeat(np.asarray(h_0, f32)[1][:, None], _BS, 1)
    c1 = np.repeat(np.asarray(c_0, f32)[0][:, None], _BS, 1)
    c2 = np.repeat(np.asarray(c_0, f32)[1][:, None], _BS, 1)

    enc = np.asarray(enc_outputs, f32)[:, 0, :n_nodes, :]   # [B, n, E]
    meanx = np.asarray(mean, f32)[:, :fut, :n_nodes, :]     # [B, fut, n, O]

    in_maps = []
    for k in range(_NCORES):
        bs = slice(k * _BS, (k + 1) * _BS)
        enc_c = np.ascontiguousarray(
            enc[bs].transpose(1, 2, 0)).reshape(n_nodes * 64, _BS)
        mean_c = np.ascontiguousarray(
            meanx[bs].transpose(2, 3, 0, 1)).reshape(n_nodes * 16, 8 * fut)
        in_maps.append({
            "enc": enc_c, "meanx": mean_c, "w1": w1, "w2": w2,
            "wr1": wr1, "wr2": wr2, "sT": sT, "h01": h01, "h02": h02,
            "c1i": c1, "c2i": c2, "onesr": ones8, "zerosr": zeros8,
        })
    return in_maps


def run_sharded(n_nodes, in_maps, trace=False, fut=_FUT, debug=False, ncores=_NCORES):
    from concourse.bass_utils import run_bass_kernel_spmd
    key = (n_nodes, fut, debug)
    if key not in _cache:
        _cache[key] = _build(n_nodes, fut=fut, debug=debug)
    nc = _cache[key]
    return run_bass_kernel_spmd(nc, in_maps, list(range(ncores)), trace=trace)


def kernel(enc_outputs, mean, s, h_0, c_0,
           W_ih_l0, W_hh_l0, b_ih_l0, b_hh_l0, W_hr_l0,
           W_ih_l1, W_hh_l1, b_ih_l1, b_hh_l1, W_hr_l1, future,
           n_nodes=None):
    assert int(future) == _FUT
    if n_nodes is None:
        n_nodes = int(np.asarray(enc_outputs).shape[2])
    in_maps = _prep_inputs(enc_outputs, mean, s, h_0, c_0,
                           W_ih_l0, W_hh_l0, b_ih_l0, b_hh_l0, W_hr_l0,
                           W_ih_l1, W_hh_l1, b_ih_l1, b_hh_l1, W_hr_l1, n_nodes)
    res = run_sharded(n_nodes, in_maps)
    outs = []
    for k in range(_NCORES):
        o = res.results[k]["outp"].reshape(n_nodes, 16, _FUT, _BS)
        outs.append(o.transpose(3, 2, 0, 1))      # [b, t, n, o]
    return np.concatenate(outs, axis=0).astype(np.float32)
